# revision 26
# baseline (speedup 1.0000x reference)
"""Contrastive soft-DTW loss kernel for Trainium2 (8 NeuronCores).

Fully on-device soft-DTW, 32 anchor/candidate pairs per core.

1) Cost-matrix phase (baseline orientation): PE computes D~ = D/gamma per
   pair in 128-row blocks; each block is evicted (compute-engine copy) and
   scattered by chunk into asrow, a DRAM layout keyed by wavefront
   iteration: asrow[r + c*K, 32c+p, :] = D~[row r, pair p, chunk-c cols].
   The DP then fetches one [128, K*CW] slab per superstep with a single
   gpsimd-issued DMA (no SP sequencer cost, uniform offsets).
2) Decoupled stabilizer: the exact hard-DTW row M (tensor_tensor_scan
   min/add) runs one superstep AHEAD of the soft recurrence, which is kept
   in s-domain (s = e^{M-R~}, validated on the real data: max gap 25.1 <<
   88). All exp arguments {Mp[j-1], Mp[j], Mc[j-1]} + q are M-relative and
   >= 0 up to rounding, so est in (0,1]; the per-row soft chain is only
   u0/u1 -> w -> s-scan. Exp-prep (q, a1/a2/aP, exp) is BATCHED over the
   K=8 rows of a superstep into a handful of wide ops.
3) K-grouped boundary shuttles, split by domain: one psh matmul + landing
   per superstep for the M boundaries (plus a bias matmul injecting the
   chunk-0 border) and one for the s boundaries (chunk-0 border is 0 for
   free). State rows live in a 2K-slot double-half ring so all APs stay
   arithmetic. PE/Act boundary cost amortizes K-fold and the M-chain never
   waits on the soft chain.
4) Extraction by DMA dump: for iterations g >= GMIN the [128,202] state
   slot is DMA'd (gpsimd software DGE) to DRAM; the host picks M* and s*
   per pair and finishes r = gamma*(M* - ln s*) plus the tiny contrastive
   reduction.

Host fallback (pure numpy, same algorithm) guards against device failure.
"""

import os

import numpy as np

LAST_RESULTS = None  # BassKernelResults of the last device run (for test.py)

NW, NG, NF = 16, 5, 10
STEP = 1 + NG + NF          # 16
T, DIM = 400, 64
GAMMA = 5.0
BIG = 1e10
BIGS2 = float(np.float32(1e5))   # finite +inf stand-in, small enough that
                                 # fp32 rounding noise on it (~8e-3) cannot
                                 # compound in warm-up garbage regions
NCORES = 8
PPC = (NW * STEP) // NCORES  # 32 pairs per core
WPC = PPC // STEP            # 2 writers per core
KAUG = DIM + 2               # 66
CH = 4                       # column chunks
CW = T // CH                 # 100 columns per chunk
NROW = T - 1                 # DP rows 2..400 -> row iterations r=0..398
K = 8                        # rows per superstep (boundary batch)
NIT = NROW + (CH - 1) * K    # 423 logical iterations
NSS = (NIT + K - 1) // K     # 53 supersteps
NITP = NSS * K               # 424 padded iterations
GMIN = 198                   # first iteration any pair can extract at
NDUMP = NITP - GMIN          # 226 dumped state slots
SLOT = 2 * (CW + 1)          # 202 state columns per row slot

_BLOCKS = []
_c = 1
while _c <= T - 1:
    _nb = min(128, T - _c)
    _BLOCKS.append((_c, _nb))
    _c += _nb


def _patch_drain():
    """Split the tile-context teardown Drain's semaphore waits across
    separate sync-engine nops (this walrus rejects multi-wait Drains)."""
    import concourse.tile as tile
    from concourse import mybir
    from concourse.vector_clock import ScopedClock

    if getattr(tile.TileContext, "_drain_patched", False):
        return
    MAXW = 1

    def _drain_and_barrier(self, tick_clock, wait_clock):
        nc = self.nc
        probe = nc.sync.nop(nofuse=True)
        wait_clock.add_sem_waits(
            probe.ins, ScopedClock({None: tick_clock.global_clock})
        )
        si = probe.ins.sync_info
        waits = list(si.on_wait) if si is not None else []
        ups = list(si.on_update) if si is not None else []
        if len(waits) > MAXW:
            probe.ins.sync_info = mybir.SyncInfo(on_wait=waits[:MAXW], on_update=ups)
            rest = waits[MAXW:]
            for k in range(0, len(rest), MAXW):
                n = nc.sync.nop(nofuse=True)
                n.ins.sync_info = mybir.SyncInfo(
                    on_wait=rest[k:k + MAXW], on_update=[]
                )
        nc.sync.drain()
        nc.all_engine_barrier()
        assert self.sems is not None
        popped = nc._tile_sem_poison_stack.pop()
        assert popped is self._sem_poison
        nc.clear_and_free_semaphores(list(self.sems.allocated().values()))
        nc.all_engine_barrier()

    tile.TileContext._drain_and_barrier = _drain_and_barrier
    tile.TileContext._drain_patched = True


def _split_bir_waits(bir_bytes):
    """This walrus rejects engine instructions carrying more than one
    embedded sync-wait. Hoist all but one wait of every instruction onto
    injected same-engine NoOps placed just before it."""
    import json

    bir = json.loads(bir_bytes)
    ctr = [0]

    def fix_block(insts):
        out = []
        for ins in insts:
            si = ins.get("sync_info")
            waits = (si or {}).get("on_wait") or []
            if len(waits) > 1:
                for wv in waits[:-1]:
                    ctr[0] += 1
                    out.append({
                        "debug": ins.get("debug", 0),
                        "engine": ins["engine"],
                        "ins": [], "outs": [],
                        "name": f"I-SW{ctr[0]}",
                        "opcode": "NoOp",
                        "sync_info": {"on_update": [], "on_wait": [wv]},
                    })
                si["on_wait"] = [waits[-1]]
            out.append(ins)
        return out

    def walk(o):
        if isinstance(o, dict):
            if isinstance(o.get("instructions"), list):
                o["instructions"] = fix_block(o["instructions"])
            for v in o.values():
                walk(v)
        elif isinstance(o, list):
            for v in o:
                walk(v)

    walk(bir)
    return json.dumps(bir).encode()


def _patch_compile():
    from concourse import bass2jax

    if getattr(bass2jax, "_split_waits_patched", False):
        return
    orig = bass2jax.compile_bir_kernel

    def wrapped(bir, *a, **k):
        return orig(_split_bir_waits(bir), *a, **k)

    bass2jax.compile_bir_kernel = wrapped
    bass2jax._split_waits_patched = True


def _build_bass():
    import concourse.bass as bass
    import concourse.tile as tile
    from concourse import mybir
    from concourse.ap import AP

    _patch_drain()
    _patch_compile()
    f32 = mybir.dt.float32
    op = mybir.AluOpType
    act = mybir.ActivationFunctionType

    nc = bass.Bass()
    aT = nc.dram_tensor("aT", [WPC, KAUG, T], f32, kind="ExternalInput")
    bT = nc.dram_tensor("bT", [PPC, KAUG, T], f32, kind="ExternalInput")
    pre = nc.dram_tensor("pre", [CH, 32, SLOT], f32, kind="ExternalInput")
    pshift = nc.dram_tensor("pshift", [128, 128], f32, kind="ExternalInput")
    dumps = nc.dram_tensor("dumps", [NDUMP, 128, SLOT], f32,
                           kind="ExternalOutput")
    asrow = nc.dram_tensor("asrow", [NITP, 128, CW], f32, kind="Internal")

    with tile.TileContext(nc) as tc:
        with tc.tile_pool(name="pp", bufs=1) as pp, \
             tc.tile_pool(name="mmp", bufs=2, space="PSUM") as mmp, \
             tc.tile_pool(name="stg", bufs=3) as stgp, \
             tc.tile_pool(name="dr", bufs=3) as drp, \
             tc.tile_pool(name="bpp", bufs=2, space="PSUM") as bpp:

            # ---------------- persistent tiles ----------------
            STATE = pp.tile([128, 2 * K * SLOT], f32, tag="STATE", name="STATE")
            TST8 = pp.tile([128, 3 * CW * K], f32, tag="TST8", name="TST8")
            EST8s = [pp.tile([128, 3 * CW * K], f32, tag="EST8a", name="EST8a"),
                     pp.tile([128, 3 * CW * K], f32, tag="EST8b", name="EST8b")]
            Q8 = pp.tile([128, CW * K], f32, tag="Q8", name="Q8")
            MN = pp.tile([128, CW], f32, tag="MN", name="MN")
            U0 = pp.tile([128, CW], f32, tag="U0", name="U0")
            U1 = pp.tile([128, CW], f32, tag="U1", name="U1")
            WT = pp.tile([128, CW], f32, tag="WT", name="WT")
            psh = pp.tile([128, 128], f32, tag="psh", name="psh")
            biasM = pp.tile([1, 128], f32, tag="biasM", name="biasM")
            selM = pp.tile([1, K], f32, tag="selM", name="selM")
            ones = pp.tile([128, CW], f32, tag="ones", name="ones")
            rA = []
            for w in range(WPC):
                t_ = pp.tile([KAUG, T], f32, tag=f"rA{w}", name=f"rA{w}")
                nc.sync.dma_start(out=t_, in_=aT[w])
                rA.append(t_)
            ltB = []
            for p in range(PPC):
                t_ = pp.tile([KAUG, T], f32, tag=f"ltB{p}", name=f"ltB{p}")
                nc.sync.dma_start(out=t_, in_=bT[p])
                ltB.append(t_)
            nc.sync.dma_start(out=psh, in_=pshift[:, :])

            # constants / garbage hygiene
            nc.vector.memset(biasM[0:1, 0:32], BIGS2)
            nc.vector.memset(biasM[0:1, 32:128], 0.0)
            nc.vector.memset(selM, 1.0)
            nc.vector.memset(ones, 1.0)
            for sl in range(2 * K):
                nc.vector.memset(STATE[:, sl * SLOT:sl * SLOT + CW + 1], BIGS2)
                nc.vector.memset(STATE[:, sl * SLOT + CW + 1:(sl + 1) * SLOT], 0.0)
            # asrow rows never covered by evictions (warm-up / drained)
            for g in list(range((CH - 1) * K)) + list(range(NROW, NITP)):
                nc.sync.dma_start(out=asrow[g], in_=ones)

            # ---------------- phase 1: D~ blocks into asrow ----------------
            def do_block(c0, nb):
                for p in range(PPC):
                    w = p // STEP
                    ps = mmp.tile([128, T], f32, tag="ps")
                    nc.tensor.matmul(ps[:nb], rA[w][:, c0:c0 + nb],
                                     ltB[p][:, :], start=True, stop=True)
                    st_ = stgp.tile([128, T], f32, tag="stg")
                    nc.scalar.copy(out=st_[:nb], in_=ps[:nb])
                    # one DMA scatters all 4 chunk segments of this block
                    # (gpsimd SWDGE: Pool is idle during phase 1, and this
                    # keeps the SP queue free for the DP-phase loads/dumps)
                    nc.gpsimd.dma_start(
                        out=AP(asrow, (c0 - 1) * 128 * CW + p * CW,
                               [[128 * CW, nb],
                                [K * 128 * CW + 32 * CW, CH], [1, CW]]),
                        in_=AP(st_.tensor, st_.offset,
                               [[st_.ap[0][0], nb], [CW, CH], [1, CW]]))

            # ---------------- phase 2: pipelined wavefront DP --------------
            sstride = STATE.ap[0][0]
            qstride = Q8.ap[0][0]
            tstride = TST8.ap[0][0]
            mlast = SLOT * (K - 1) + 1

            def land(dst_off, psum):
                nc.scalar.copy(out=STATE[:, dst_off:dst_off + mlast:SLOT],
                               in_=psum)

            def load_drow8(s):
                d8 = drp.tile([128, K * CW], f32, tag="drow8")
                nc.sync.dma_start(
                    out=d8,
                    in_=AP(asrow, s * K * 128 * CW,
                           [[CW, 128], [128 * CW, K], [1, CW]]))
                return d8

            def m_block(s, d8):
                """Hard-DP rows + batched exp-prep for superstep s."""
                h = s % 2
                base = h * K * SLOT
                obase = (1 - h) * K * SLOT
                if s >= 1 and s <= CH - 1:
                    c = s
                    pcol = ((c - 1) % 2) * K * SLOT + (K - 1) * SLOT
                    nc.sync.dma_start(
                        out=STATE[32 * c:32 * c + 32, pcol:pcol + SLOT],
                        in_=pre[c])
                for t in range(K):
                    slot = base + t * SLOT
                    pslot = (base + (t - 1) * SLOT) if t > 0 \
                        else (obase + (K - 1) * SLOT)
                    Mp = STATE[:, pslot:pslot + CW + 1]
                    nc.vector.tensor_tensor(out=MN, in0=Mp[:, 0:CW],
                                            in1=Mp[:, 1:CW + 1], op=op.min)
                    nc.vector.tensor_tensor_scan(
                        out=STATE[:, slot + 1:slot + CW + 1], data0=MN,
                        data1=d8[:, t * CW:(t + 1) * CW],
                        initial=STATE[:, slot:slot + 1],
                        op0=op.min, op1=op.add)
                # shuttle M boundaries of this superstep's rows
                psM = bpp.tile([128, K], f32, tag="psM")
                nc.tensor.matmul(psM, psh[:, :],
                                 STATE[:, base + CW:base + CW + mlast:SLOT],
                                 start=True, stop=False)
                nc.tensor.matmul(psM, biasM[:, :], selM[:, :],
                                 start=False, stop=True)
                # batched prep: q, a1/a2 windows, aP, exp
                nc.gpsimd.tensor_tensor(
                    out=Q8, in0=d8,
                    in1=AP(STATE.tensor, STATE.offset + base + 1,
                           [[sstride, 128], [SLOT, K], [1, CW]]),
                    op=op.subtract)
                # t = 0 (Mp slot is in the other half -> separate op)
                nc.vector.tensor_tensor(
                    out=AP(TST8.tensor, TST8.offset,
                           [[tstride, 128], [CW, 2], [1, CW]]),
                    in0=AP(STATE.tensor,
                           STATE.offset + obase + (K - 1) * SLOT,
                           [[sstride, 128], [1, 2], [1, CW]]),
                    in1=AP(Q8.tensor, Q8.offset,
                           [[qstride, 128], [0, 2], [1, CW]]),
                    op=op.add)
                # t = 1..K-1 batched
                nc.vector.tensor_tensor(
                    out=AP(TST8.tensor, TST8.offset + 3 * CW,
                           [[tstride, 128], [3 * CW, K - 1], [CW, 2], [1, CW]]),
                    in0=AP(STATE.tensor, STATE.offset + base,
                           [[sstride, 128], [SLOT, K - 1], [1, 2], [1, CW]]),
                    in1=AP(Q8.tensor, Q8.offset + CW,
                           [[qstride, 128], [CW, K - 1], [0, 2], [1, CW]]),
                    op=op.add)
                # aP for all K rows
                nc.gpsimd.tensor_tensor(
                    out=AP(TST8.tensor, TST8.offset + 2 * CW,
                           [[tstride, 128], [3 * CW, K], [1, CW]]),
                    in0=AP(STATE.tensor, STATE.offset + base,
                           [[sstride, 128], [SLOT, K], [1, CW]]),
                    in1=AP(Q8.tensor, Q8.offset,
                           [[qstride, 128], [CW, K], [1, CW]]),
                    op=op.add)
                nc.scalar.activation(out=EST8s[s % 2][:, :], in_=TST8[:, :],
                                     func=act.Exp, scale=-1.0)
                return psM

            def s_block(s):
                """Soft-DP rows + dumps + s-boundary shuttle, superstep s."""
                h = s % 2
                base = h * K * SLOT
                obase = (1 - h) * K * SLOT
                EST8 = EST8s[s % 2]
                for t in range(K):
                    g = s * K + t
                    slot = base + t * SLOT
                    pslot = (base + (t - 1) * SLOT) if t > 0 \
                        else (obase + (K - 1) * SLOT)
                    e0 = t * 3 * CW
                    nc.gpsimd.tensor_tensor(
                        out=U0, in0=STATE[:, pslot + CW + 1:pslot + 2 * CW + 1],
                        in1=EST8[:, e0:e0 + CW], op=op.mult)
                    nc.vector.tensor_tensor(
                        out=U1, in0=STATE[:, pslot + CW + 2:pslot + SLOT],
                        in1=EST8[:, e0 + CW:e0 + 2 * CW], op=op.mult)
                    nc.gpsimd.tensor_tensor(out=WT, in0=U0, in1=U1, op=op.add)
                    nc.vector.tensor_tensor_scan(
                        out=STATE[:, slot + CW + 2:slot + SLOT],
                        data0=EST8[:, e0 + 2 * CW:e0 + 3 * CW], data1=WT,
                        initial=STATE[:, slot + CW + 1:slot + CW + 2],
                        op0=op.mult, op1=op.add)
                    if g >= GMIN:
                        nc.sync.dma_start(
                            out=dumps[g - GMIN],
                            in_=STATE[:, slot:slot + SLOT])
                    if t == K - 2:
                        # shuttle slots 0..K-2 early, off the boundary chain
                        psSa = bpp.tile([128, K - 1], f32, tag="psSa")
                        nc.tensor.matmul(
                            psSa, psh[:, :],
                            STATE[:, base + 2 * CW + 1:
                                  base + 2 * CW + 1 + SLOT * (K - 2) + 1:SLOT],
                            start=True, stop=True)
                psSb = bpp.tile([128, 1], f32, tag="psSb")
                nc.tensor.matmul(
                    psSb, psh[:, :],
                    STATE[:, base + 2 * CW + 1 + SLOT * (K - 1):
                          base + 2 * CW + 1 + SLOT * (K - 1) + 1],
                    start=True, stop=True)
                return psSa, psSb

            # chunk-0 row-1 preload into the last slot of half 1
            nc.sync.dma_start(
                out=STATE[0:32, (2 * K - 1) * SLOT:2 * K * SLOT], in_=pre[0])

            # block 0 first: it covers the asrow rows the first supersteps
            # read, so the DP prologue can overlap the rest of phase 1
            do_block(*_BLOCKS[0])
            d8 = [None] * NSS
            d8[0] = load_drow8(0)
            d8[1] = load_drow8(1)
            for (c0, nb) in _BLOCKS[1:]:
                do_block(c0, nb)

            # prologue: prime the M pipeline for superstep 0
            psM = m_block(0, d8[0])
            psS = None
            for s in range(NSS):
                if psS is not None:
                    # s-boundaries for superstep s's rows (split landing)
                    b0 = (s % 2) * K * SLOT + CW + 1
                    nc.scalar.copy(
                        out=STATE[:, b0:b0 + SLOT * (K - 2) + 1:SLOT],
                        in_=psS[0])
                    nc.scalar.copy(
                        out=STATE[:, b0 + SLOT * (K - 1):
                                  b0 + SLOT * (K - 1) + 1],
                        in_=psS[1])
                s_block_ps = s_block(s)
                if s + 1 < NSS:
                    if s + 2 < NSS:
                        d8[s + 2] = load_drow8(s + 2)
                    # M-boundaries for superstep s+1's rows
                    land(((s + 1) % 2) * K * SLOT, psM)
                    psM = m_block(s + 1, d8[s + 1])
                psS = s_block_ps
    return nc


def _prep_inputs(A, B, la, lb):
    """Build per-core input maps. A/B: [256, T, DIM] fp32."""
    P = A.shape[0]
    asq = np.sum(A * A, axis=-1)
    bsq = np.sum(B * B, axis=-1)

    anchors = A[::STEP]
    asq_w = asq[::STEP]
    aTm = np.empty((NW, KAUG, T), np.float32)
    aTm[:, :DIM] = np.transpose(anchors * np.float32(-2.0 / GAMMA), (0, 2, 1))
    aTm[:, DIM] = asq_w / np.float32(GAMMA)
    aTm[:, DIM + 1] = 1.0

    bTm = np.empty((P, KAUG, T), np.float32)
    bTm[:, :DIM] = np.transpose(B, (0, 2, 1))
    bTm[:, DIM] = 1.0
    bTm[:, DIM + 1] = bsq / np.float32(GAMMA)

    # row 1 of the DP: R~[1, j] = cumsum_j D~[0, j-1]; stabilizer M1 := R~1
    d0 = (asq[:, 0:1] + bsq - 2.0 * np.einsum("pd,ptd->pt", A[:, 0], B)) \
        / np.float32(GAMMA)
    r1 = np.cumsum(d0.astype(np.float32), axis=1, dtype=np.float32)  # [P, T]

    pshift = np.zeros((128, 128), np.float32)
    for k in range(96):
        pshift[k, k + 32] = 1.0

    in_maps = []
    for core in range(NCORES):
        sl = slice(core * PPC, (core + 1) * PPC)
        wsl = slice(core * WPC, (core + 1) * WPC)
        r1c = r1[sl]
        prec = np.empty((CH, 32, SLOT), np.float32)
        for c in range(CH):
            prec[c, :, 0] = BIGS2 if c == 0 else r1c[:, c * CW - 1]
            prec[c, :, 1:CW + 1] = r1c[:, c * CW:(c + 1) * CW]
            prec[c, :, CW + 1] = 0.0 if c == 0 else 1.0
            prec[c, :, CW + 2:] = 1.0
        in_maps.append({
            "aT": np.ascontiguousarray(aTm[wsl]),
            "bT": np.ascontiguousarray(bTm[sl]),
            "pre": prec,
            "pshift": pshift,
        })
    return in_maps


def _device_r(A, B, la, lb):
    from concourse.bass_utils import run_bass_kernel_spmd

    in_maps = _prep_inputs(A, B, la, lb)
    nc = _build_bass()
    kw = {}
    if os.environ.get("KERNEL_TRACE", "") == "1":
        kw = dict(trace=True, tmpdir=os.environ.get("KERNEL_TRACE_DIR") or None)
    res = run_bass_kernel_spmd(nc, in_maps, core_ids=list(range(NCORES)), **kw)
    global LAST_RESULTS
    LAST_RESULTS = res
    r = np.empty(A.shape[0], np.float32)
    for core in range(NCORES):
        out = res.results[core]["dumps"]          # [NDUMP, 128, SLOT]
        sl = slice(core * PPC, (core + 1) * PPC)
        lbc, lac = lb[sl], la[sl]
        cstar = (lbc - 1) // CW
        kstar = lbc - cstar * CW                  # 1..CW
        part = 32 * cstar + np.arange(PPC)
        g = (lac - 2) + cstar * K
        mstar = out[g - GMIN, part, kstar]
        sstar = out[g - GMIN, part, CW + 1 + kstar]
        r[sl] = np.float32(GAMMA) * (mstar - np.log(sstar))
    return r


# ---------------- host fallback (same algorithm, numpy) ----------------

def _host_r(A, B, la, lb):
    P = A.shape[0]
    f = np.float32
    asq = np.sum(A * A, axis=-1)
    bsq = np.sum(B * B, axis=-1)
    cross = np.einsum("ptd,psd->pts", A, B, optimize=True)
    Dt = ((asq[:, :, None] + bsq[:, None, :] - 2.0 * cross)
          / np.float32(GAMMA)).astype(f)
    R1 = np.cumsum(Dt[:, 0, :], axis=1, dtype=f)
    Mp = np.concatenate([np.full((P, 1), BIGS2, f), R1], 1)
    sp = np.ones((P, T + 1), f)
    sp[:, 0] = 0.0
    Mc = np.empty_like(Mp)
    st = np.empty_like(sp)
    rM = np.zeros(P, f)
    rS = np.ones(P, f)
    for i in range(2, T + 1):
        Drow = Dt[:, i - 1, :]
        mn = np.minimum(Mp[:, :-1], Mp[:, 1:]).astype(f)
        Mc[:, 0] = BIGS2
        c = Mc[:, 0].copy()
        for j in range(T):
            c = (np.minimum(mn[:, j], c) + Drow[:, j]).astype(f)
            Mc[:, j + 1] = c
        q = (Drow - Mc[:, 1:]).astype(f)
        e1 = np.exp(-(Mp[:, :-1] + q)).astype(f)
        e2 = np.exp(-(Mp[:, 1:] + q)).astype(f)
        eP = np.exp(-(Mc[:, :-1] + q)).astype(f)
        w = (sp[:, :-1] * e1 + sp[:, 1:] * e2).astype(f)
        st[:, 0] = 0.0
        c = st[:, 0].copy()
        for j in range(T):
            c = (eP[:, j] * c + w[:, j]).astype(f)
            st[:, j + 1] = c
        sel = la == i
        if sel.any():
            idx = np.where(sel)[0]
            rM[idx] = Mc[idx, lb[idx]]
            rS[idx] = st[idx, lb[idx]]
        Mp, Mc = Mc, Mp
        sp, st = st, sp
    return np.float32(GAMMA) * (rM - np.log(rS))


def kernel(data, margin, lens):
    data = np.asarray(data, dtype=np.float32)
    margin = np.asarray(margin, dtype=np.float32)
    lens = np.asarray(lens)

    batch = data.reshape(NW, STEP, T, DIM)
    blens = lens.reshape(NW, STEP)
    A = np.ascontiguousarray(
        np.broadcast_to(batch[:, :1], batch.shape).reshape(NW * STEP, T, DIM),
        dtype=np.float32)
    B = np.ascontiguousarray(batch.reshape(NW * STEP, T, DIM),
                             dtype=np.float32)
    la = np.broadcast_to(blens[:, :1], blens.shape).reshape(-1).astype(np.int64)
    lb = blens.reshape(-1).astype(np.int64)

    if (la < 200).any() or (lb < 2).any() or (la > T).any() or (lb > T).any():
        r = _host_r(A, B, la, lb)
    else:
        try:
            r = _device_r(A, B, la, lb)
        except Exception as e:  # pragma: no cover - device fallback
            import traceback
            traceback.print_exc()
            print(f"[kernel] device path failed ({e!r}); falling back to host")
            r = _host_r(A, B, la, lb)

    dists = (r / (la + lb).astype(np.float32)).reshape(NW, STEP)
    dist_aa = dists[:, 0:1]
    lk1 = dists[:, 1:1 + NG] - dist_aa
    lk2 = np.maximum(margin[0] - (dists[:, 1 + NG:1 + NG + NF] - dist_aa), 0.0)
    nz = (lk1 != 0).sum(axis=1) + (lk2 != 0).sum(axis=1) + 1
    lv = (lk1.sum(axis=1) + lk2.sum(axis=1)) / nz
    return np.float32(lv.mean())


# revision 30
# speedup vs baseline: 1.2248x; 1.2248x over previous
"""Contrastive soft-DTW loss kernel for Trainium2 (8 NeuronCores).

Fully on-device soft-DTW, 32 anchor/candidate pairs per core.

1) Cost-matrix phase (baseline orientation): PE computes D~ = D/gamma per
   pair in 128-row blocks; each block is evicted (compute-engine copy) and
   scattered by chunk into asrow, a DRAM layout keyed by wavefront
   iteration: asrow[r + c*K, 32c+p, :] = D~[row r, pair p, chunk-c cols].
   The DP then fetches one [128, K*CW] slab per superstep with a single
   gpsimd-issued DMA (no SP sequencer cost, uniform offsets).
2) Decoupled stabilizer: the exact hard-DTW row M (tensor_tensor_scan
   min/add) runs one superstep AHEAD of the soft recurrence, which is kept
   in s-domain (s = e^{M-R~}, validated on the real data: max gap 25.1 <<
   88). All exp arguments {Mp[j-1], Mp[j], Mc[j-1]} + q are M-relative and
   >= 0 up to rounding, so est in (0,1]; the per-row soft chain is only
   u0/u1 -> w -> s-scan. Exp-prep (q, a1/a2/aP, exp) is BATCHED over the
   K=8 rows of a superstep into a handful of wide ops.
3) K-grouped boundary shuttles, split by domain: one psh matmul + landing
   per superstep for the M boundaries (plus a bias matmul injecting the
   chunk-0 border) and one for the s boundaries (chunk-0 border is 0 for
   free). State rows live in a 2K-slot double-half ring so all APs stay
   arithmetic. PE/Act boundary cost amortizes K-fold and the M-chain never
   waits on the soft chain.
4) Extraction by DMA dump: for iterations g >= GMIN the [128,202] state
   slot is DMA'd (gpsimd software DGE) to DRAM; the host picks M* and s*
   per pair and finishes r = gamma*(M* - ln s*) plus the tiny contrastive
   reduction.

Host fallback (pure numpy, same algorithm) guards against device failure.
"""

import os

import numpy as np

LAST_RESULTS = None  # BassKernelResults of the last device run (for test.py)

NW, NG, NF = 16, 5, 10
STEP = 1 + NG + NF          # 16
T, DIM = 400, 64
GAMMA = 5.0
BIG = 1e10
BIGS2 = float(np.float32(1e5))   # finite +inf stand-in, small enough that
                                 # fp32 rounding noise on it (~8e-3) cannot
                                 # compound in warm-up garbage regions
NCORES = 8
PPC = (NW * STEP) // NCORES  # 32 pairs per core
WPC = PPC // STEP            # 2 writers per core
KAUG = DIM + 2               # 66
CH = 4                       # column chunks
CW = T // CH                 # 100 columns per chunk
NROW = T - 1                 # DP rows 2..400 -> row iterations r=0..398
K = 8                        # rows per superstep (boundary batch)
NIT = NROW + (CH - 1) * K    # 423 logical iterations
NSS = (NIT + K - 1) // K     # 53 supersteps
NITP = NSS * K               # 424 padded iterations
GMIN = 198                   # first iteration any pair can extract at
NDUMP = NITP - GMIN          # 226 dumped state slots
SLOT = 2 * (CW + 1)          # 202 state columns per row slot

_BLOCKS = []
_c = 1
while _c <= T - 1:
    _nb = min(128, T - _c)
    _BLOCKS.append((_c, _nb))
    _c += _nb


def _patch_drain():
    """Split the tile-context teardown Drain's semaphore waits across
    separate sync-engine nops (this walrus rejects multi-wait Drains)."""
    import concourse.tile as tile
    from concourse import mybir
    from concourse.vector_clock import ScopedClock

    if getattr(tile.TileContext, "_drain_patched", False):
        return
    MAXW = 1

    def _drain_and_barrier(self, tick_clock, wait_clock):
        nc = self.nc
        probe = nc.sync.nop(nofuse=True)
        wait_clock.add_sem_waits(
            probe.ins, ScopedClock({None: tick_clock.global_clock})
        )
        si = probe.ins.sync_info
        waits = list(si.on_wait) if si is not None else []
        ups = list(si.on_update) if si is not None else []
        if len(waits) > MAXW:
            probe.ins.sync_info = mybir.SyncInfo(on_wait=waits[:MAXW], on_update=ups)
            rest = waits[MAXW:]
            for k in range(0, len(rest), MAXW):
                n = nc.sync.nop(nofuse=True)
                n.ins.sync_info = mybir.SyncInfo(
                    on_wait=rest[k:k + MAXW], on_update=[]
                )
        nc.sync.drain()
        nc.all_engine_barrier()
        assert self.sems is not None
        popped = nc._tile_sem_poison_stack.pop()
        assert popped is self._sem_poison
        nc.clear_and_free_semaphores(list(self.sems.allocated().values()))
        nc.all_engine_barrier()

    tile.TileContext._drain_and_barrier = _drain_and_barrier
    tile.TileContext._drain_patched = True


def _split_bir_waits(bir_bytes):
    """This walrus rejects engine instructions carrying more than one
    embedded sync-wait. Hoist all but one wait of every instruction onto
    injected same-engine NoOps placed just before it."""
    import json

    bir = json.loads(bir_bytes)
    ctr = [0]

    def fix_block(insts):
        out = []
        for ins in insts:
            si = ins.get("sync_info")
            waits = (si or {}).get("on_wait") or []
            if len(waits) > 1:
                for wv in waits[:-1]:
                    ctr[0] += 1
                    out.append({
                        "debug": ins.get("debug", 0),
                        "engine": ins["engine"],
                        "ins": [], "outs": [],
                        "name": f"I-SW{ctr[0]}",
                        "opcode": "NoOp",
                        "sync_info": {"on_update": [], "on_wait": [wv]},
                    })
                si["on_wait"] = [waits[-1]]
            out.append(ins)
        return out

    def walk(o):
        if isinstance(o, dict):
            if isinstance(o.get("instructions"), list):
                o["instructions"] = fix_block(o["instructions"])
            for v in o.values():
                walk(v)
        elif isinstance(o, list):
            for v in o:
                walk(v)

    walk(bir)
    return json.dumps(bir).encode()


def _patch_compile():
    from concourse import bass2jax

    if getattr(bass2jax, "_split_waits_patched", False):
        return
    orig = bass2jax.compile_bir_kernel

    def wrapped(bir, *a, **k):
        return orig(_split_bir_waits(bir), *a, **k)

    bass2jax.compile_bir_kernel = wrapped
    bass2jax._split_waits_patched = True


def _build_bass():
    import concourse.bass as bass
    import concourse.tile as tile
    from concourse import mybir
    from concourse.ap import AP

    _patch_drain()
    _patch_compile()
    f32 = mybir.dt.float32
    op = mybir.AluOpType
    act = mybir.ActivationFunctionType

    nc = bass.Bass()
    aT = nc.dram_tensor("aT", [WPC, KAUG, T], f32, kind="ExternalInput")
    bT = nc.dram_tensor("bT", [PPC, KAUG, T], f32, kind="ExternalInput")
    pre = nc.dram_tensor("pre", [CH, 32, SLOT], f32, kind="ExternalInput")
    pshift = nc.dram_tensor("pshift", [128, 128], f32, kind="ExternalInput")
    dumps = nc.dram_tensor("dumps", [NDUMP, 128, SLOT], f32,
                           kind="ExternalOutput")
    asrow = nc.dram_tensor("asrow", [NITP, 128, CW], f32, kind="Internal")

    with tile.TileContext(nc) as tc:
        with tc.tile_pool(name="pp", bufs=1) as pp, \
             tc.tile_pool(name="mmp", bufs=3, space="PSUM") as mmp, \
             tc.tile_pool(name="stg", bufs=3) as stgp, \
             tc.tile_pool(name="dr", bufs=3) as drp, \
             tc.tile_pool(name="bpp", bufs=2, space="PSUM") as bpp:

            # ---------------- persistent tiles ----------------
            STATE = pp.tile([128, 2 * K * SLOT], f32, tag="STATE", name="STATE")
            TST8 = pp.tile([128, 3 * CW * K], f32, tag="TST8", name="TST8")
            EST8s = [pp.tile([128, 3 * CW * K], f32, tag="EST8a", name="EST8a"),
                     pp.tile([128, 3 * CW * K], f32, tag="EST8b", name="EST8b")]
            Q8 = pp.tile([128, CW * K], f32, tag="Q8", name="Q8")
            MN = pp.tile([128, CW], f32, tag="MN", name="MN")
            UU = pp.tile([128, 2 * CW], f32, tag="UU", name="UU")
            WT = pp.tile([128, CW], f32, tag="WT", name="WT")
            psh = pp.tile([128, 128], f32, tag="psh", name="psh")
            biasM = pp.tile([1, 128], f32, tag="biasM", name="biasM")
            selM = pp.tile([1, K], f32, tag="selM", name="selM")
            ones = pp.tile([128, CW], f32, tag="ones", name="ones")
            rA = []
            for w in range(WPC):
                t_ = pp.tile([KAUG, T], f32, tag=f"rA{w}", name=f"rA{w}")
                nc.sync.dma_start(out=t_, in_=aT[w])
                rA.append(t_)
            ltB = []
            for p in range(PPC):
                t_ = pp.tile([KAUG, T], f32, tag=f"ltB{p}", name=f"ltB{p}")
                nc.sync.dma_start(out=t_, in_=bT[p])
                ltB.append(t_)
            nc.sync.dma_start(out=psh, in_=pshift[:, :])

            # constants / garbage hygiene
            nc.vector.memset(biasM[0:1, 0:32], BIGS2)
            nc.vector.memset(biasM[0:1, 32:128], 0.0)
            nc.vector.memset(selM, 1.0)
            nc.vector.memset(ones, 1.0)
            for sl in range(2 * K):
                nc.vector.memset(STATE[:, sl * SLOT:sl * SLOT + CW + 1], BIGS2)
                nc.vector.memset(STATE[:, sl * SLOT + CW + 1:(sl + 1) * SLOT], 0.0)
            # asrow rows never covered by evictions (warm-up / drained)
            for g in list(range((CH - 1) * K)) + list(range(NROW, NITP)):
                nc.sync.dma_start(out=asrow[g], in_=ones)

            # ---------------- phase 1: D~ blocks into asrow ----------------
            def do_block(c0, nb):
                for p in range(PPC):
                    w = p // STEP
                    ps = mmp.tile([128, T], f32, tag="ps")
                    nc.tensor.matmul(ps[:nb], rA[w][:, c0:c0 + nb],
                                     ltB[p][:, :], start=True, stop=True)
                    st_ = stgp.tile([128, T], f32, tag="stg")
                    nc.scalar.copy(out=st_[:nb], in_=ps[:nb])
                    # one DMA scatters all 4 chunk segments of this block
                    # (gpsimd SWDGE: Pool is idle during phase 1, and this
                    # keeps the SP queue free for the DP-phase loads/dumps)
                    nc.gpsimd.dma_start(
                        out=AP(asrow, (c0 - 1) * 128 * CW + p * CW,
                               [[128 * CW, nb],
                                [K * 128 * CW + 32 * CW, CH], [1, CW]]),
                        in_=AP(st_.tensor, st_.offset,
                               [[st_.ap[0][0], nb], [CW, CH], [1, CW]]))

            # ---------------- phase 2: pipelined wavefront DP --------------
            sstride = STATE.ap[0][0]
            qstride = Q8.ap[0][0]
            tstride = TST8.ap[0][0]
            mlast = SLOT * (K - 1) + 1

            def land(dst_off, psum):
                nc.scalar.copy(out=STATE[:, dst_off:dst_off + mlast:SLOT],
                               in_=psum)

            def load_drow8(s):
                d8 = drp.tile([128, K * CW], f32, tag="drow8")
                nc.sync.dma_start(
                    out=d8,
                    in_=AP(asrow, s * K * 128 * CW,
                           [[CW, 128], [128 * CW, K], [1, CW]]))
                return d8

            def m_block(s, d8):
                """Hard-DP rows + batched exp-prep for superstep s."""
                h = s % 2
                base = h * K * SLOT
                obase = (1 - h) * K * SLOT
                if s >= 1 and s <= CH - 1:
                    c = s
                    pcol = ((c - 1) % 2) * K * SLOT + (K - 1) * SLOT
                    nc.sync.dma_start(
                        out=STATE[32 * c:32 * c + 32, pcol:pcol + SLOT],
                        in_=pre[c])
                for t in range(K):
                    slot = base + t * SLOT
                    pslot = (base + (t - 1) * SLOT) if t > 0 \
                        else (obase + (K - 1) * SLOT)
                    Mp = STATE[:, pslot:pslot + CW + 1]
                    nc.vector.tensor_tensor(out=MN, in0=Mp[:, 0:CW],
                                            in1=Mp[:, 1:CW + 1], op=op.min)
                    nc.vector.tensor_tensor_scan(
                        out=STATE[:, slot + 1:slot + CW + 1], data0=MN,
                        data1=d8[:, t * CW:(t + 1) * CW],
                        initial=STATE[:, slot:slot + 1],
                        op0=op.min, op1=op.add)
                # shuttle M boundaries of this superstep's rows
                psM = bpp.tile([128, K], f32, tag="psM")
                nc.tensor.matmul(psM, psh[:, :],
                                 STATE[:, base + CW:base + CW + mlast:SLOT],
                                 start=True, stop=False)
                nc.tensor.matmul(psM, biasM[:, :], selM[:, :],
                                 start=False, stop=True)
                # batched prep: q, a1/a2 windows, aP, exp
                nc.gpsimd.tensor_tensor(
                    out=Q8, in0=d8,
                    in1=AP(STATE.tensor, STATE.offset + base + 1,
                           [[sstride, 128], [SLOT, K], [1, CW]]),
                    op=op.subtract)
                # t = 0 (Mp slot is in the other half -> separate op)
                nc.gpsimd.tensor_tensor(
                    out=AP(TST8.tensor, TST8.offset,
                           [[tstride, 128], [CW, 2], [1, CW]]),
                    in0=AP(STATE.tensor,
                           STATE.offset + obase + (K - 1) * SLOT,
                           [[sstride, 128], [1, 2], [1, CW]]),
                    in1=AP(Q8.tensor, Q8.offset,
                           [[qstride, 128], [0, 2], [1, CW]]),
                    op=op.add)
                # t = 1..K-1 batched
                nc.gpsimd.tensor_tensor(
                    out=AP(TST8.tensor, TST8.offset + 3 * CW,
                           [[tstride, 128], [3 * CW, K - 1], [CW, 2], [1, CW]]),
                    in0=AP(STATE.tensor, STATE.offset + base,
                           [[sstride, 128], [SLOT, K - 1], [1, 2], [1, CW]]),
                    in1=AP(Q8.tensor, Q8.offset + CW,
                           [[qstride, 128], [CW, K - 1], [0, 2], [1, CW]]),
                    op=op.add)
                # aP for all K rows
                nc.gpsimd.tensor_tensor(
                    out=AP(TST8.tensor, TST8.offset + 2 * CW,
                           [[tstride, 128], [3 * CW, K], [1, CW]]),
                    in0=AP(STATE.tensor, STATE.offset + base,
                           [[sstride, 128], [SLOT, K], [1, CW]]),
                    in1=AP(Q8.tensor, Q8.offset,
                           [[qstride, 128], [CW, K], [1, CW]]),
                    op=op.add)
                nc.scalar.activation(out=EST8s[s % 2][:, :], in_=TST8[:, :],
                                     func=act.Exp, scale=-1.0)
                return psM

            def s_block(s):
                """Soft-DP rows + dumps + s-boundary shuttle, superstep s."""
                h = s % 2
                base = h * K * SLOT
                obase = (1 - h) * K * SLOT
                EST8 = EST8s[s % 2]
                for t in range(K):
                    g = s * K + t
                    slot = base + t * SLOT
                    pslot = (base + (t - 1) * SLOT) if t > 0 \
                        else (obase + (K - 1) * SLOT)
                    e0 = t * 3 * CW
                    # u0|u1 in one 200-wide op: sp[j-1]/sp[j] windows vs e1|e2
                    nc.vector.tensor_tensor(
                        out=AP(UU.tensor, UU.offset,
                               [[UU.ap[0][0], 128], [CW, 2], [1, CW]]),
                        in0=AP(STATE.tensor, STATE.offset + pslot + CW + 1,
                               [[sstride, 128], [1, 2], [1, CW]]),
                        in1=AP(EST8.tensor, EST8.offset + e0,
                               [[EST8.ap[0][0], 128], [CW, 2], [1, CW]]),
                        op=op.mult)
                    nc.vector.tensor_tensor(out=WT, in0=UU[:, 0:CW],
                                            in1=UU[:, CW:2 * CW], op=op.add)
                    nc.vector.tensor_tensor_scan(
                        out=STATE[:, slot + CW + 2:slot + SLOT],
                        data0=EST8[:, e0 + 2 * CW:e0 + 3 * CW], data1=WT,
                        initial=STATE[:, slot + CW + 1:slot + CW + 2],
                        op0=op.mult, op1=op.add)
                    if g >= GMIN:
                        nc.sync.dma_start(
                            out=dumps[g - GMIN],
                            in_=STATE[:, slot:slot + SLOT])
                psS = bpp.tile([128, K], f32, tag="psS")
                nc.tensor.matmul(
                    psS, psh[:, :],
                    STATE[:, base + 2 * CW + 1:base + 2 * CW + 1 + mlast:SLOT],
                    start=True, stop=True)
                return psS

            # chunk-0 row-1 preload into the last slot of half 1
            nc.sync.dma_start(
                out=STATE[0:32, (2 * K - 1) * SLOT:2 * K * SLOT], in_=pre[0])

            # block 0 first: it covers the asrow rows the first supersteps
            # read, so the DP prologue can overlap the rest of phase 1
            do_block(*_BLOCKS[0])
            d8 = [None] * NSS
            d8[0] = load_drow8(0)
            d8[1] = load_drow8(1)
            for (c0, nb) in _BLOCKS[1:]:
                do_block(c0, nb)

            # prologue: prime the M pipeline for superstep 0
            psM = m_block(0, d8[0])
            psS = None
            for s in range(NSS):
                if psS is not None:
                    # s-boundaries for superstep s's rows
                    land((s % 2) * K * SLOT + CW + 1, psS)
                s_block_ps = s_block(s)
                if s + 1 < NSS:
                    if s + 2 < NSS:
                        d8[s + 2] = load_drow8(s + 2)
                    # M-boundaries for superstep s+1's rows
                    land(((s + 1) % 2) * K * SLOT, psM)
                    psM = m_block(s + 1, d8[s + 1])
                psS = s_block_ps
    return nc


def _prep_inputs(A, B, la, lb):
    """Build per-core input maps. A/B: [256, T, DIM] fp32."""
    P = A.shape[0]
    asq = np.sum(A * A, axis=-1)
    bsq = np.sum(B * B, axis=-1)

    anchors = A[::STEP]
    asq_w = asq[::STEP]
    aTm = np.empty((NW, KAUG, T), np.float32)
    aTm[:, :DIM] = np.transpose(anchors * np.float32(-2.0 / GAMMA), (0, 2, 1))
    aTm[:, DIM] = asq_w / np.float32(GAMMA)
    aTm[:, DIM + 1] = 1.0

    bTm = np.empty((P, KAUG, T), np.float32)
    bTm[:, :DIM] = np.transpose(B, (0, 2, 1))
    bTm[:, DIM] = 1.0
    bTm[:, DIM + 1] = bsq / np.float32(GAMMA)

    # row 1 of the DP: R~[1, j] = cumsum_j D~[0, j-1]; stabilizer M1 := R~1
    d0 = (asq[:, 0:1] + bsq - 2.0 * np.einsum("pd,ptd->pt", A[:, 0], B)) \
        / np.float32(GAMMA)
    r1 = np.cumsum(d0.astype(np.float32), axis=1, dtype=np.float32)  # [P, T]

    pshift = np.zeros((128, 128), np.float32)
    for k in range(96):
        pshift[k, k + 32] = 1.0

    in_maps = []
    for core in range(NCORES):
        sl = slice(core * PPC, (core + 1) * PPC)
        wsl = slice(core * WPC, (core + 1) * WPC)
        r1c = r1[sl]
        prec = np.empty((CH, 32, SLOT), np.float32)
        for c in range(CH):
            prec[c, :, 0] = BIGS2 if c == 0 else r1c[:, c * CW - 1]
            prec[c, :, 1:CW + 1] = r1c[:, c * CW:(c + 1) * CW]
            prec[c, :, CW + 1] = 0.0 if c == 0 else 1.0
            prec[c, :, CW + 2:] = 1.0
        in_maps.append({
            "aT": np.ascontiguousarray(aTm[wsl]),
            "bT": np.ascontiguousarray(bTm[sl]),
            "pre": prec,
            "pshift": pshift,
        })
    return in_maps


def _device_r(A, B, la, lb):
    from concourse.bass_utils import run_bass_kernel_spmd

    in_maps = _prep_inputs(A, B, la, lb)
    nc = _build_bass()
    kw = {}
    if os.environ.get("KERNEL_TRACE", "") == "1":
        kw = dict(trace=True, tmpdir=os.environ.get("KERNEL_TRACE_DIR") or None)
    res = run_bass_kernel_spmd(nc, in_maps, core_ids=list(range(NCORES)), **kw)
    global LAST_RESULTS
    LAST_RESULTS = res
    r = np.empty(A.shape[0], np.float32)
    for core in range(NCORES):
        out = res.results[core]["dumps"]          # [NDUMP, 128, SLOT]
        sl = slice(core * PPC, (core + 1) * PPC)
        lbc, lac = lb[sl], la[sl]
        cstar = (lbc - 1) // CW
        kstar = lbc - cstar * CW                  # 1..CW
        part = 32 * cstar + np.arange(PPC)
        g = (lac - 2) + cstar * K
        mstar = out[g - GMIN, part, kstar]
        sstar = out[g - GMIN, part, CW + 1 + kstar]
        r[sl] = np.float32(GAMMA) * (mstar - np.log(sstar))
    return r


# ---------------- host fallback (same algorithm, numpy) ----------------

def _host_r(A, B, la, lb):
    P = A.shape[0]
    f = np.float32
    asq = np.sum(A * A, axis=-1)
    bsq = np.sum(B * B, axis=-1)
    cross = np.einsum("ptd,psd->pts", A, B, optimize=True)
    Dt = ((asq[:, :, None] + bsq[:, None, :] - 2.0 * cross)
          / np.float32(GAMMA)).astype(f)
    R1 = np.cumsum(Dt[:, 0, :], axis=1, dtype=f)
    Mp = np.concatenate([np.full((P, 1), BIGS2, f), R1], 1)
    sp = np.ones((P, T + 1), f)
    sp[:, 0] = 0.0
    Mc = np.empty_like(Mp)
    st = np.empty_like(sp)
    rM = np.zeros(P, f)
    rS = np.ones(P, f)
    for i in range(2, T + 1):
        Drow = Dt[:, i - 1, :]
        mn = np.minimum(Mp[:, :-1], Mp[:, 1:]).astype(f)
        Mc[:, 0] = BIGS2
        c = Mc[:, 0].copy()
        for j in range(T):
            c = (np.minimum(mn[:, j], c) + Drow[:, j]).astype(f)
            Mc[:, j + 1] = c
        q = (Drow - Mc[:, 1:]).astype(f)
        e1 = np.exp(-(Mp[:, :-1] + q)).astype(f)
        e2 = np.exp(-(Mp[:, 1:] + q)).astype(f)
        eP = np.exp(-(Mc[:, :-1] + q)).astype(f)
        w = (sp[:, :-1] * e1 + sp[:, 1:] * e2).astype(f)
        st[:, 0] = 0.0
        c = st[:, 0].copy()
        for j in range(T):
            c = (eP[:, j] * c + w[:, j]).astype(f)
            st[:, j + 1] = c
        sel = la == i
        if sel.any():
            idx = np.where(sel)[0]
            rM[idx] = Mc[idx, lb[idx]]
            rS[idx] = st[idx, lb[idx]]
        Mp, Mc = Mc, Mp
        sp, st = st, sp
    return np.float32(GAMMA) * (rM - np.log(rS))


def kernel(data, margin, lens):
    data = np.asarray(data, dtype=np.float32)
    margin = np.asarray(margin, dtype=np.float32)
    lens = np.asarray(lens)

    batch = data.reshape(NW, STEP, T, DIM)
    blens = lens.reshape(NW, STEP)
    A = np.ascontiguousarray(
        np.broadcast_to(batch[:, :1], batch.shape).reshape(NW * STEP, T, DIM),
        dtype=np.float32)
    B = np.ascontiguousarray(batch.reshape(NW * STEP, T, DIM),
                             dtype=np.float32)
    la = np.broadcast_to(blens[:, :1], blens.shape).reshape(-1).astype(np.int64)
    lb = blens.reshape(-1).astype(np.int64)

    if (la < 200).any() or (lb < 2).any() or (la > T).any() or (lb > T).any():
        r = _host_r(A, B, la, lb)
    else:
        try:
            r = _device_r(A, B, la, lb)
        except Exception as e:  # pragma: no cover - device fallback
            import traceback
            traceback.print_exc()
            print(f"[kernel] device path failed ({e!r}); falling back to host")
            r = _host_r(A, B, la, lb)

    dists = (r / (la + lb).astype(np.float32)).reshape(NW, STEP)
    dist_aa = dists[:, 0:1]
    lk1 = dists[:, 1:1 + NG] - dist_aa
    lk2 = np.maximum(margin[0] - (dists[:, 1 + NG:1 + NG + NF] - dist_aa), 0.0)
    nz = (lk1 != 0).sum(axis=1) + (lk2 != 0).sum(axis=1) + 1
    lv = (lk1.sum(axis=1) + lk2.sum(axis=1)) / nz
    return np.float32(lv.mean())
